# revision 35
# baseline (speedup 1.0000x reference)
"""Multi-head (per-task) 2-layer MLP classifier for Trainium2, 8 NeuronCores.

Strategy: expert-parallel with host-side dispatch. Rows of x are grouped by
task_id on the host (the all-to-all "dispatch"); core t gets all rows whose
task_id == t, zero-padded to a fixed PAD columns, pre-transposed to x^T
[D, PAD]. Each core then runs a dense 2-layer MLP for its own head only:

    H^T = relu(W1^T x^T + b1)        [H, PAD]   (psum: out=W1.T@xT, lhsT=W1)
    Y^T = W2^T H^T + b2              [C, PAD]   (lhsT=W2, rhs=H^T)

Everything stays "transposed" (feature dim on partitions, batch on the free
dim) so both matmuls chain without any on-device transpose, and both biases
are per-partition vectors. The host scatters Y^T columns back to the
original row order.

This computes each row through exactly one head (the reference computes all
8 heads and selects via one-hot -> 8x the FLOPs) and reads each expert's
weights from HBM exactly once across the whole chip.

Perf notes (8xNC_v3, NTFF profile; 54.7us -> ~51.5us -> ~50.2us typ):
- Measured timeline (max core): 0-7.2us walrus preamble ($E[4] host
  trigger ~2.8us + two S[2] rendezvous rounds + per-queue TENSOR_LOAD
  queue-init, all fixed); first DMAs issue at kernel entry ~7.25us but
  their completion SEMS fire only ~11.3-12.3us (desc-gen 0.6-0.85us +
  ring/engine pipe; data packets start flowing ~8.2us); PE stream
  ~33.1us of pure matmul cycles (512-col matmul start-pitch is EXACTLY
  216ns = 512cy@2.37GHz, ldweights fully hidden); tail ~2.2-2.5us.
- exec_time_ns ends ~0.5us after the LAST OUTPUT DMA PACKET: the NEFF
  epilogue's 256-sem sweep is mostly EXCLUDED from the measured window
  (revising the earlier ~7us-epilogue theory). Only the last-output
  path matters at the end. The teardown diet (KMM_FASTDRAIN) stays.
- PE warmup: 10 big (512-free) warms bridge entry->gate. The p-state
  ramp needs HIGH-MAC-RATE busy: bridging with 64-free micro-warms
  parked the clock governor at ~2.17GHz for the ENTIRE stream (pitch
  235ns vs 216ns, +3.7us) — only full-width warms are safe. A gap
  between warm-end and gate resets the ramp (~0.6us hiccup + ~2us
  re-ramp at 427ns/matmul), so erring on +1 warm is cheaper.
- w1/xt DMAs alternate across BOTH HWDGE rings in consumption order
  (w1_k and xt_k0 on opposite rings, flipping each k): a single-ring
  w1 stream starves the chunk-0 k-sweep 0.6-2.7us on slow cores
  because ring issue pitch is ~0.65-0.7us/DMA (desc-gen-bound,
  size-independent) and the wait legalizer merges sem waits so
  k-groups gate on w1 tiles ~2 ahead.
- Output: per-chunk CONTIGUOUS DRAM tensors yt0/yt1/yt2 (a column
  slice of one [C,pad] tensor wrote 100x256B strided packets). Tail
  floor ~2.2us = ps2 drain 0.3 + desc-gen ~0.8 (FIXED per DMA, not
  row-proportional) + 100 latency-bound packets ~1.0 + sem margin 0.5.
- PSUM drains (bias+relu) alternate DVE/ACT (GpSimd cannot touch
  PSUM). ~280-320ns each regardless of width (fixed PSUM-read setup).
- Tail chunk L1 runs in m-groups of 2 (KMM_TAILHALF/KMM_TAILG=2,
  quarters): each group's k=7 stops land progressively, so drains +
  L2 matmuls overlap later groups' L1 instead of one drain-paced
  chain at the end. Quarters beat halves (~0.3us) and beat no split
  (~0.4-0.7us); g=1 would serialize ldweights (~1us/m-group, 8us
  tail). Applied ONLY to the last chunk — for chunk 0 a partial
  m-sweep doubles the early DMA demand rate and starves. The tail's
  ps2 drain also splits rows [0:64]/[64:100] across DVE and ACT in
  parallel (engine APs need 32-aligned base partitions — a 50/50
  split crashes codegen).
- The first-DMA completion sems really do fire at ~11-12.3us
  (verified: first real LDWEIGHTS evt_wait 3.8us on S[w1_0]); the
  4-byte sem packets seen at 8.4-9.4us are the SWDGE bias/w2
  completions, NOT the ring tiles. Cutting warms below ~10 on that
  false signal measured +0.5-2us (idle gap 8.4->11 + full re-ramp).
- Dead ends, measured this round: micro-warms (p-state poison, above);
  tail chunk processed mid-stream (final 512-wide write transfers
  204KB serially, worse); tail write split across both rings (desc-gen
  is fixed-cost, keep-warm dummies add overhead, net worse); walrus
  --max-sem-num (bass allocates kernel sems at 151+ itself; and the
  sweep barely counts in the window anyway); DMA directly from PSUM
  (bass asserts src in SBUF/DRAM); late-xt tiles over SWDGE (single
  queue ~50-60GB/s, +3us; plain dma_start can't fan across the 4
  SWDGE queues — only gather/scatter take queue_num).
- Run-to-run variance is ±0.5-1us with occasional +2-3us outliers:
  single-core mid-stream delivery stalls (all 8 cores' rings contend
  for the shared DMA-engine/HBM pool in the 8-20us window at ~2TB/s
  aggregate). Environmental; not addressable from one SPMD program.
- Dead ends from earlier rounds: fp8 fails the 2e-2 gate (3.5e-2;
  DoubleRow=2x only helps if BOTH operands stay plain fp8 — any hi/lo
  compensation burns the pair slot and is >= bf16 cost); walrus
  --enable-ldw-opt rejects tile-emitted InstLdweights; SWDGE for the
  critical output; removing the final drain (~1us worse).
- Remaining structure (don't re-chase): ~0.4-0.5us instruction-fetch
  bubbles at chunk boundaries (1KB iqueue pages); the tail L2 is
  drain/ldweights-paced (~1.2us after last-chunk L1); preamble+DMA
  pipe = ~12.3us fixed front-end. bf16 PE floor for 1088 cols is
  ~33.1us; per-core load is data-given (max task count 1082).
"""

import os

import numpy as np

import concourse.bacc as bacc
import concourse.bass as bass
import concourse.mybir as mybir
import concourse.tile as tile
from concourse.bass_utils import run_bass_kernel_spmd
from concourse.vector_clock import ScopedClock

# Problem constants (nn_MultiHeadClassifier: T tasks, 2-layer MLP heads)
T = 8          # tasks == cores
D = 1024       # d_model
HID = 1024     # hidden
C = 100        # classes
B = 8192       # batch
P = 128        # partitions
KD = D // P    # k-tiles in layer-1 contraction
KH = HID // P  # k-tiles in layer-2 contraction

# Per-core padded batch. Task counts for the graded inputs (seed 0) max
# out at 1082 -> pad 1088; _run grows this automatically if a different
# distribution needs more.
PAD_DEFAULT = 1088

_MM_DTYPES = {
    "f32": mybir.dt.float32,
    "f32r": mybir.dt.float32r,
    "bf16": mybir.dt.bfloat16,
}


def _lean_drain_and_barrier(self, tick_clock, wait_clock):
    """Teardown diet: both all_engine_barriers and the semaphore clear
    are dropped — the walrus NEFF epilogue has its own ordered
    rendezvous chain on S[2] (each queue waits S[2]==k then increments,
    preceded by a walrus-emitted per-queue ring DRAIN) before its full
    256-sem zeroing sweep, so no kernel-side barrier is needed and the
    sweep makes the kernel-side sem clear redundant.

    KMM_FASTDRAIN (default on): the kernel-side drain is emitted WITHOUT
    its global-clock waits, so queues reach the rendezvous at their last
    real instruction and most of the output flush overlaps the ~7us
    sweep. Output consistency holds by a wide margin: the final yt DMA
    lands long before the sweeps finish, walrus's own ring drains cover
    the rings, and the runtime drains them again at teardown; the sweep
    zeroing a DMA-lane semaphore that a late completion then bumps is
    benign — nothing waits on those sems afterward and the next
    execution's preamble re-clears the whole range. (Removing the drain
    instruction entirely measured ~1us WORSE with higher variance.)"""
    drain_inst = self.nc.sync.drain()
    if os.environ.get("KMM_FASTDRAIN", "1") != "1":
        wait_clock.add_sem_waits(
            drain_inst.ins, ScopedClock({None: tick_clock.global_clock})
        )
    popped = self.nc._tile_sem_poison_stack.pop()
    assert popped is self._sem_poison


tile.TileContext._drain_and_barrier = _lean_drain_and_barrier


def _bvo_ldwopt(tmpdir, inp="bir.json", outp="file.neff", arch=None, *,
                dve_root=None):
    """bass_utils.bir_verify_and_optimise with --enable-ldw-opt=true
    (upstream hardcodes false). Elides redundant LDWEIGHTS for
    back-to-back matmuls sharing a stationary operand."""
    import concourse.bass_utils as _bu
    from pathlib import Path as _Path
    cmd = [
        _bu.get_walrus_driver(),
        "--pass",
        ",".join([
            "birverifier", "runtime_memory_reservation", "lower_act",
            "lower_dve", "lower_ap_offset", "codegen", "neff_packager",
        ]),
        "-i", inp,
        "--neff-output-filename", outp,
        "--enable-birsim=true",
        "--mem-mode=physical",
        "--policy=0",
        "--enable-ldw-opt=true",
        "--assign-static-dmas-to-sp=false",
        f"--dram-page-size={_bu.aot_getenv('NEURON_SCRATCHPAD_PAGE_SIZE', '256')}",
        "--enable-neff-debug-info=true",
        "--jobs", "8",
        *_bu.get_walrus_args(
            _bu.get_bir_arch(tmpdir, inp) if arch is None else arch,
            tmpdir, dve_root=dve_root,
        ),
    ]
    result = _bu.run_command(cmd, cwd=tmpdir)
    if result is not None:
        (_Path(tmpdir) / "log.txt").write_text(result.stdout)
    return f"{tmpdir}/{outp}"


# Dead end, kept for reference: walrus rejects the tile-emitted explicit
# InstLdweights with "not compatible with LDW optimization".
if os.environ.get("KMM_LDWOPT", "0") == "1":
    import concourse.bass_utils as _bu_mod
    _bu_mod.bir_verify_and_optimise = _bvo_ldwopt


def _bvo_extra_flags(extra):
    """Wrap bass_utils.bir_verify_and_optimise, appending walrus flags."""
    import concourse.bass_utils as _bu
    import subprocess as _sp
    orig_run = _bu.run_command

    def patched_run(cmd, **kw):
        if cmd and "walrus_driver" in str(cmd[0]):
            cmd = list(cmd) + extra
        return orig_run(cmd, **kw)

    _bu.run_command = patched_run


# Experiment: cap walrus's semaphore allocation. The NTFF "useful window"
# (what exec_time_ns measures) ends when the epilogue's ascending per-queue
# 256-sem zeroing sweep clears the last semaphore id the kernel used;
# fewer/lower sem ids end the window sooner.
if os.environ.get("KMM_SEMCAP"):
    _bvo_extra_flags([f"--max-sem-num={os.environ['KMM_SEMCAP']}"])


def _chunks(total, size=512):
    out, o = [], 0
    while o < total:
        c = min(size, total - o)
        out.append((o, c))
        o += c
    return out


def build_program(pad, mm_dtype="bf16"):
    """One SPMD NeuronCore program: dense 2-layer MLP on [D, pad] x^T."""
    mm_dt = _MM_DTYPES[mm_dtype]
    f32 = mybir.dt.float32
    # Tensors consumed by the matmuls carry the matmul dtype end-to-end
    # (walrus requires fp32r-consumed buffers to be *produced* as fp32r).
    io_dt = mm_dt

    def mm(ap):
        return ap.bitcast(mm_dt) if ap.dtype != mm_dt else ap

    nc = bacc.Bacc()
    # Hybrid-precision L1 contraction (KMM_FP8PAIRS, default OFF): put
    # 2*pairs k-slices in ONE fp8e4 DoubleRow matmul per (chunk, m) —
    # the PE packs 2 k-slices per instruction at bf16 speed — while the
    # rest stay bf16. MEASURED at 1 pair (minimum coverage): 47509ns
    # (saves ~2.9us) but rel err 2.1417e-2 — 7% OVER the 2e-2 gate
    # (error scales as sqrt(coverage); full-fp8 extrapolates to 4.3e-2;
    # W-side/x-side split is ~55/45 so no single-side fix reaches 2e-2,
    # and every compensation scheme costs exactly the 2x DoubleRow
    # saves: the pair slot must hold the correction). Kept for a future
    # relaxed gate; enable with KMM_FP8PAIRS=1.
    fp8_pairs = int(os.environ.get("KMM_FP8PAIRS", "0"))
    assert fp8_pairs * 2 <= KD
    kd_b = KD - 2 * fp8_pairs  # bf16 k-slices
    fp8 = mybir.dt.float8e4
    d_split = fp8_pairs * 2 * P  # first d rows in fp8
    if fp8_pairs:
        w1f = nc.dram_tensor("w1f", [P, fp8_pairs * 2 * HID], fp8,
                             kind="ExternalInput")
        xtf = nc.dram_tensor("xtf", [P, fp8_pairs * 2 * pad], fp8,
                             kind="ExternalInput")
    xt = nc.dram_tensor("xt", [kd_b * P, pad], io_dt, kind="ExternalInput")
    w1 = nc.dram_tensor("w1", [kd_b * P, HID], io_dt, kind="ExternalInput")
    b1 = nc.dram_tensor("b1", [P, KH], f32, kind="ExternalInput")
    w2 = nc.dram_tensor("w2", [HID, P], io_dt, kind="ExternalInput")
    b2 = nc.dram_tensor("b2", [C, 1], f32, kind="ExternalInput")
    chunks = _chunks(pad)
    # Per-chunk output tensors: each chunk's [C, cw] block is CONTIGUOUS in
    # DRAM (rows of cw*4B at stride cw*4B), so the final write coalesces
    # into large packets instead of the 256B-per-row strided writes a
    # column-slice of a single [C, pad] tensor produces (measured: the
    # 64-col tail chunk wrote 100 x 256B packets over ~1.3us).
    yts = [nc.dram_tensor(f"yt{ci}", [C, cw], f32, kind="ExternalOutput")
           for ci, (_, cw) in enumerate(chunks)]
    # (KMM_TAILSPLIT=1 — splitting the tail write across both rings with
    # keep-warm dummies — was measured neutral-to-worse: DMA desc-gen is
    # ~0.8us FIXED per instruction regardless of row count, so the split
    # only parallelizes ~0.5us of packets while the dummies add overhead.)
    tailsplit = os.environ.get("KMM_TAILSPLIT", "0") == "1"
    ydummy = (nc.dram_tensor("ydummy", [1, 4], f32, kind="Internal")
              if tailsplit else None)

    w1_t = w1.rearrange("(k p) h -> k p h", p=P)
    xt_t = xt.rearrange("(k p) b -> k p b", p=P)
    w2_t = w2.rearrange("(k p) c -> k p c", p=P)

    with tile.TileContext(nc) as tc:
        with (
            tc.tile_pool(name="weights", bufs=1) as wpool,
            tc.tile_pool(name="acts", bufs=1) as apool,
            tc.tile_pool(name="ps", bufs=8, space="PSUM") as pspool,
            tc.tile_pool(name="outs", bufs=3) as opool,
        ):
            # DMA plan. The critical first tiles (w1 k=0 halves, xt chunk-0
            # k-tiles) ride the SP HWDGE ring, issued FIRST and interleaved
            # so PE's chunk-0 k-sweep consumes each (w1_k, xt_k0) as it
            # lands; odd w1 k-tiles ride the Activation HWDGE ring (which
            # starts late: the RELU act-table load is queued ahead of it).
            # Small/late tensors (biases, w2, output) ride SWDGE (gpsimd).
            b1_sb = wpool.tile([P, KH], f32, name="b1", tag="b1")
            nc.gpsimd.dma_start(out=b1_sb[:], in_=b1[:])
            b2_sb = wpool.tile([C, 1], f32, name="b2", tag="b2")
            nc.gpsimd.dma_start(out=b2_sb[:], in_=b2[:])

            w2_all = wpool.tile([P, KH, P], io_dt, name="w2_all", tag="w2_all")
            nc.gpsimd.dma_start(
                out=w2_all[:],
                in_=w2.rearrange("(k p) c -> p k c", p=P),
            )
            w2_sb = [w2_all[:, k, :] for k in range(KH)]


            # xt tile plan: one tile per (k, big-chunk); a trailing small
            # chunk (pad % 512) is folded into the previous tile so it
            # needs no extra DMA. (Merging each k's xt into one [P, pad]
            # tile was measured WORSE: pad*2B rows cross the 2KB
            # descriptor boundary and split into an inefficient 2nd
            # descriptor per row.) chunk_src[ci] = (tile index, offset).
            tiles = []          # (start, width)
            chunk_src = []      # ci -> (tile_idx, offset)
            for (o, cw) in chunks:
                if tiles and cw < 128:
                    ti, (ts, tw) = len(tiles) - 1, tiles[-1]
                    chunk_src.append((ti, o - ts))
                    tiles[ti] = (ts, tw + cw)
                else:
                    chunk_src.append((len(tiles), 0))
                    tiles.append((o, cw))

            w1_sb = [None] * KD
            xt_sb = [[None] * len(tiles) for _ in range(KD)]

            def load_w1(k, q):
                w1_sb[k] = wpool.tile([P, HID], io_dt, name=f"w1_{k}",
                                      tag=f"w1_{k}")
                q.dma_start(out=w1_sb[k][:], in_=w1_t[k, :, :])

            def load_xt(k, ti, q):
                ts, tw = tiles[ti]
                t = wpool.tile([P, tw], io_dt, name=f"xt_{k}_{ti}",
                               tag=f"xt_{k}_{ti}")
                q.dma_start(out=t[:], in_=xt_t[k, :, ts:ts + tw])
                xt_sb[k][ti] = t

            # DMA order (trace-measured pipe latencies: the sync/SP ring's
            # first DMA lands ~4.0us after issue, the scalar/ACT ring's
            # ~5.0us — sync arms ~1us earlier). The first-matmul gate is
            # (w1_0, xt_00): put w1_0 FIRST on sync (lands ~11.6us) and
            # xt_00 FIRST on scalar (~11.9us) so neither serializes behind
            # the other on one ring. Splitting w1_0 finer is useless: ring
            # issue pitch (~0.7us/DMA descriptor-gen) dominates, and the
            # wait-legalization hoists the k-group's waits anyway.
            # Interleave w1/xt across BOTH rings in consumption order. With
            # w1 all on one ring (0.65-0.7us/DMA issue pitch) the chunk-0
            # k-sweep starves 0.6-1.5us at ~16-17us on most cores: the
            # wait legalizer merges sem waits so k-group k gates on w1
            # tiles ~2 ahead of k, and a single ring delivers w1_7 only by
            # ~17.3us. Alternating (w1_k, xt_k0) pairs across the rings
            # delivers each stream at 2x rate in need order.
            if fp8_pairs:
                w1f_sb = wpool.tile([P, fp8_pairs * 2 * HID], fp8,
                                    name="w1f", tag="w1f")
                nc.scalar.dma_start(out=w1f_sb[:], in_=w1f[:])
                xtf_sb = wpool.tile([P, fp8_pairs * 2 * pad], fp8,
                                    name="xtf", tag="xtf")
                nc.sync.dma_start(out=xtf_sb[:], in_=xtf[:])
                w1f_ap = w1f_sb[:].rearrange(
                    "p (two h) -> p two h", two=fp8_pairs * 2)
                xtf_ap = xtf_sb[:].rearrange(
                    "p (two b) -> p two b", two=fp8_pairs * 2)
            # Balanced first-group split (KMM_SPLIT0): the gate is the
            # max over both rings' first transfers; with w1_0 (256KB) on
            # one ring and xt_00 (128KB) on the other it is bound by the
            # 256KB ring. Halving BOTH across BOTH rings makes each
            # ring's first two items 192KB, and k0's m-sweep consumes
            # (w1 half, xt col-half) pieces as they land.
            split0 = (os.environ.get("KMM_SPLIT0", "1") == "1"
                      and chunks[0][1] == 512)
            if split0:
                w1_0a = wpool.tile([P, 512], io_dt, name="w1_0a",
                                   tag="w1_0a")
                nc.scalar.dma_start(out=w1_0a[:], in_=w1_t[0, :, 0:512])
                xt_00a = wpool.tile([P, 256], io_dt, name="xt_00a",
                                    tag="xt_00a")
                nc.scalar.dma_start(out=xt_00a[:], in_=xt_t[0, :, 0:256])
                w1_0b = wpool.tile([P, 512], io_dt, name="w1_0b",
                                   tag="w1_0b")
                nc.sync.dma_start(out=w1_0b[:], in_=w1_t[0, :, 512:1024])
                xt_00b = wpool.tile([P, 256], io_dt, name="xt_00b",
                                    tag="xt_00b")
                nc.sync.dma_start(out=xt_00b[:], in_=xt_t[0, :, 256:512])
            for k in range(kd_b):
                if k == 0 and split0:
                    continue
                load_w1(k, nc.scalar if k % 2 == 0 else nc.sync)
                load_xt(k, 0, nc.sync if k % 2 == 0 else nc.scalar)
            # xt tiles for chunks >= 1 are not consumed until ~26us; route
            # them over the SWDGE (gpsimd) queue so the two HWDGE rings
            # carry only w1 + chunk-0 xt (3MB instead of 4.2MB) and finish
            # chunk-0's k-tail ~1us sooner on congested cores (measured
            # 1.3us delivery stalls at ~17us with everything on HWDGE).
            # (KMM_XTSW=1 — routing them over SWDGE — was measured ~3us
            # WORSE on every core: the single SWDGE queue delivers only
            # ~50-60GB/s, starving chunks 1-2; plain dma_start cannot
            # spread across the 4 SWDGE queues.)
            xt_late_q = nc.gpsimd if os.environ.get(
                "KMM_XTSW", "0") == "1" else None
            for ti in range(1, len(tiles)):
                for k in range(kd_b):
                    load_xt(k, ti, xt_late_q or
                            (nc.sync if k % 2 == 0 else nc.scalar))

            # PE warmup: the PE clock ramps over ~3us of sustained
            # activity (cold matmuls run at ~427ns vs 216ns full speed for
            # 512-free), and the ramp is TIME-based while busy — an idle
            # gap between warm-end and the first real matmul RESETS the
            # ramp (measured: a 1.1us gap cost ~2.3us of re-ramp slowdown
            # on top of the gap itself). Bridge from PE-queue start
            # (~7.4us) to the first real matmul's DMA gate (~11.9-12.4us)
            # with 7 big warms (512-free, ~427ns cold) + fine-grained
            # micro-warms (64-free) whose overshoot granularity is small.
            # NOTE: bridging with fine-grained 64-free micro-warms was
            # measured ~3.7us WORSE: the low-MAC-rate micro-warm window
            # parks the PE clock governor in a mid p-state (~2.17GHz) for
            # the ENTIRE stream (512-free matmul pitch 235ns vs 216ns,
            # durations 379->413ns). Only big high-duty warms keep the
            # ramp going.
            warm = wpool.tile([P, 512], io_dt, name="warm", tag="warm")
            nc.vector.memset(warm[:], 0.0)
            for _ in range(int(os.environ.get("KMM_WARM", "10"))):
                pw = pspool.tile([P, 512], f32, name="ps_w", tag="ps")
                nc.tensor.matmul(out=pw[:], lhsT=mm(warm[:, 0:P]),
                                 rhs=mm(warm[:]), start=True, stop=True)

            def w1_block(k, m):
                if k == 0 and split0:
                    half = w1_0a if m < KH // 2 else w1_0b
                    mo = m % (KH // 2)
                    return half[:, mo * P:(mo + 1) * P]
                return w1_sb[k][:, m * P:(m + 1) * P]

            h_sb = [apool.tile([P, pad], io_dt, name=f"h_{m}", tag=f"h_{m}") for m in range(KH)]

            # Process the small tail chunk MID-stream (order [0, last,
            # middles]): its latency-bound output (100x256B packets, ~1us)
            # and drain-paced L2 hide under later compute, and the final
            # chunk becomes a 512-wide one whose output writes efficient
            # 2KB packets.
            order = list(range(len(chunks)))
            # (KMM_TAILMID=1 — processing the small chunk mid-stream so a
            # 512 chunk lands last — was measured NEUTRAL-to-worse: the
            # final 204KB write transfers ~2.3us serially, vs the 64-col
            # tail's latency-bound ~1.9us.)
            if len(chunks) > 2 and os.environ.get("KMM_TAILMID", "0") == "1":
                order = [0, len(chunks) - 1] + order[1:-1]
            for ci in order:
                o, cw = chunks[ci]
                ti, toff = chunk_src[ci]
                # layer 1: all KH h-tile groups resident in PSUM, k swept in
                # the middle so PE consumes (w1_k, xt_k) right as each DMA
                # lands instead of stalling a single group on the last tile.
                pss = [pspool.tile([P, 512], f32, name=f"ps_{m}", tag="ps")
                       for m in range(KH)]

                def l1_sweep(ms):
                    # fp8 DoubleRow pair(s) first (start=True zeroes the
                    # psum region), then the bf16 k-slices accumulate.
                    # k swept in the middle so PE consumes (w1_k, xt_k)
                    # right as each DMA lands instead of stalling a single
                    # group on the last tile.
                    for pr in range(fp8_pairs):
                        for m in ms:
                            nc.tensor.matmul(
                                out=pss[m][:, :cw],
                                lhsT=w1f_ap[:, 2*pr:2*pr+2,
                                            m * P:(m + 1) * P],
                                rhs=xtf_ap[:, 2*pr:2*pr+2, o:o + cw],
                                start=(pr == 0),
                                stop=False,
                                perf_mode=mybir.MatmulPerfMode.DoubleRow,
                            )
                    for k in range(kd_b):
                        for m in ms:
                            if k == 0 and ci == 0 and split0 \
                                    and not fp8_pairs:
                                # col-halves as the half-tiles land;
                                # start=True zeroes the whole 2KB psum
                                # bank region, so half B accumulates
                                # with start=False onto pending-zero.
                                nc.tensor.matmul(
                                    out=pss[m][:, 0:256],
                                    lhsT=mm(w1_block(0, m)),
                                    rhs=mm(xt_00a[:]),
                                    start=True, stop=False,
                                )
                                nc.tensor.matmul(
                                    out=pss[m][:, 256:512],
                                    lhsT=mm(w1_block(0, m)),
                                    rhs=mm(xt_00b[:]),
                                    start=False, stop=False,
                                )
                                continue
                            nc.tensor.matmul(
                                out=pss[m][:, :cw],
                                lhsT=mm(w1_block(k, m)),
                                rhs=mm(xt_sb[k][ti][:, toff:toff + cw]),
                                start=(k == 0 and not fp8_pairs
                                       and not (ci == 0 and split0)),
                                stop=(k == kd_b - 1),
                            )

                def drains(ms):
                    # PSUM -> SBUF h with bias+relu, alternating DVE/ACT so
                    # the drains take half as many op-slots of wall time.
                    # (GpSimd cannot read PSUM.)
                    for m in ms:
                        if m % 2 == 0:
                            nc.vector.tensor_scalar(
                                out=h_sb[m][:, o:o + cw],
                                in0=pss[m][:, :cw],
                                scalar1=b1_sb[:, m:m + 1],
                                scalar2=0.0,
                                op0=mybir.AluOpType.add,
                                op1=mybir.AluOpType.max,
                            )
                        else:
                            nc.scalar.activation(
                                out=h_sb[m][:, o:o + cw],
                                in_=pss[m][:, :cw],
                                func=mybir.ActivationFunctionType.Relu,
                                bias=b1_sb[:, m:m + 1],
                                scale=1.0,
                            )

                if ci == order[-1] and os.environ.get(
                        "KMM_TAILHALF", "1") == "1":
                    # Tail-critical chunk: run L1 in m-groups so early
                    # groups' k=7 stops land mid-tail — their drains and
                    # L2 matmuls overlap later groups' L1 instead of a
                    # single drain-paced chain at the end (measured ~1.2us
                    # from tail-L1 end to last L2). Group size via
                    # KMM_TAILG (2 = quarters, 4 = halves).
                    g = int(os.environ.get("KMM_TAILG", "2"))
                    for lo in range(0, KH, g):
                        l1_sweep(range(lo, lo + g))
                        drains(range(lo, lo + g))
                else:
                    l1_sweep(range(KH))
                    drains(range(KH))
                # layer 2: Y^T chunk = sum_k W2[k].T @ H^T[k] + b2
                ps2 = pspool.tile([P, 512], f32, name="ps2", tag="ps")
                for k in range(KH):
                    nc.tensor.matmul(
                        out=ps2[:, :cw],
                        lhsT=mm(w2_sb[k]),
                        rhs=mm(h_sb[k][:, o:o + cw]),
                        start=(k == 0),
                        stop=(k == KH - 1),
                    )
                ot = opool.tile([P, 512], f32, name="ot", tag="ot")
                if ci == order[-1]:
                    # Tail-critical ps2 drain: split across DVE and ACT in
                    # parallel (~halves the ~270ns drain on the chain).
                    # Split at 64: engine APs need 32-aligned base
                    # partitions.
                    c2 = 64
                    nc.vector.tensor_scalar_add(
                        out=ot[:c2, :cw],
                        in0=ps2[:c2, :cw],
                        scalar1=b2_sb[:c2, 0:1],
                    )
                    nc.scalar.activation(
                        out=ot[c2:C, :cw],
                        in_=ps2[c2:C, :cw],
                        func=mybir.ActivationFunctionType.Identity,
                        bias=b2_sb[c2:C, 0:1],
                        scale=1.0,
                    )
                else:
                    nc.vector.tensor_scalar_add(
                        out=ot[:C, :cw],
                        in0=ps2[:C, :cw],
                        scalar1=b2_sb[:, 0:1],
                    )
                if ci == order[-1] and tailsplit:
                    # Tail-critical write: split by partition rows across
                    # BOTH rings so desc-gen (~0.45us each for 50 rows) and
                    # the latency-bound 256B row packets run in parallel.
                    # The scalar ring is kept warm by the dummy DMAs below
                    # (a COLD scalar ring measured ~2us re-arm, which is
                    # why a split without keep-warm dummies lost ~0.5us).
                    c2 = (C + 1) // 2
                    nc.sync.dma_start(out=yts[ci][0:c2, :cw],
                                      in_=ot[0:c2, :cw])
                    nc.scalar.dma_start(out=yts[ci][c2:C, :cw],
                                        in_=ot[c2:C, :cw])
                else:
                    # yt rides the sync queue: each chunk's write follows
                    # the previous one on a HOT ring.
                    nc.sync.dma_start(out=yts[ci][:, :cw], in_=ot[:C, :cw])
                    if tailsplit:
                        # Keep-warm dummy on the scalar ring (~4B).
                        nc.scalar.dma_start(out=ydummy[0:1, 0:2],
                                            in_=ot[0:1, 0:2])
    return nc


def _pad_cols(a, n):
    out = np.zeros((a.shape[0], n), dtype=a.dtype)
    out[:, :a.shape[1]] = a
    return out


def _route(task_id):
    """Group rows by task. Returns (row-index list per task, counts)."""
    task_id = np.asarray(task_id)
    order = np.argsort(task_id, kind="stable")
    counts = np.bincount(task_id.astype(np.int64), minlength=T)
    offs = np.zeros(T + 1, dtype=np.int64)
    np.cumsum(counts, out=offs[1:])
    rows = [order[offs[t]:offs[t + 1]] for t in range(T)]
    return rows, counts


def _run(inputs, trace=False):
    x = np.ascontiguousarray(np.asarray(inputs["x"], dtype=np.float32))
    task_id = np.asarray(inputs["task_id"])
    W1 = np.asarray(inputs["W1"], dtype=np.float32)
    b1 = np.asarray(inputs["b1"], dtype=np.float32)
    W2 = np.asarray(inputs["W2"], dtype=np.float32)
    b2 = np.asarray(inputs["b2"], dtype=np.float32)

    mm_dtype = os.environ.get("KMM_DTYPE", "bf16")
    pad = int(os.environ.get("KMM_PAD", PAD_DEFAULT))
    rows, counts = _route(task_id)
    if counts.max() > pad:  # unexpected distribution: grow pad to fit
        pad = int(-(-int(counts.max()) // 16) * 16)

    io_np = np.float32
    if mm_dtype == "bf16":
        import ml_dtypes
        io_np = ml_dtypes.bfloat16

    fp8_pairs = int(os.environ.get("KMM_FP8PAIRS", "0"))
    d_split = fp8_pairs * 2 * P
    fp8_np = None
    if fp8_pairs:
        import ml_dtypes
        fp8_np = ml_dtypes.float8_e4m3

    def _pack_fp8(a):
        # [d_split, N] (d = slice*128 + p) -> [128, n_slices*N] fp8 with
        # per-partition layout [slice, N] (matches the [p, two, N] APs).
        n = a.shape[1]
        return np.ascontiguousarray(
            a.reshape(d_split // P, P, n).transpose(1, 0, 2).reshape(
                P, d_split // P * n)).astype(fp8_np)

    in_maps = []
    for t in range(T):
        xt = np.zeros((D, pad), dtype=np.float32)
        xt[:, :counts[t]] = x[rows[t]].T
        w1t = np.ascontiguousarray(W1[t])
        im = {
            "xt": xt[d_split:].astype(io_np),
            "w1": w1t[d_split:].astype(io_np),
            "b1": np.ascontiguousarray(b1[t].reshape(KH, P).T.astype(np.float32)),
            "w2": _pad_cols(W2[t], P).astype(io_np),
            "b2": np.ascontiguousarray(b2[t][:, None].astype(np.float32)),
        }
        if fp8_pairs:
            im["w1f"] = _pack_fp8(w1t[:d_split])
            im["xtf"] = _pack_fp8(xt[:d_split])
        in_maps.append(im)

    nc = build_program(pad, mm_dtype)
    nc.finalize()  # Bacc passes: legalize sync waits (<=1 per instruction)
    res = run_bass_kernel_spmd(
        nc, in_maps, core_ids=list(range(T)), trace=trace,
        trace_cores=list(range(T)) if trace else None,
        tmpdir=os.environ.get("KMM_TMPDIR"),
    )

    chunks = _chunks(pad)
    out = np.empty((task_id.shape[0], C), dtype=np.float32)
    for t in range(T):
        ytf = np.concatenate(
            [res.results[t][f"yt{ci}"] for ci in range(len(chunks))], axis=1)
        out[rows[t]] = ytf[:, :counts[t]].T
    return out, res


def kernel(**inputs):
    out, _ = _run(inputs, trace=False)
    return out



# revision 36
# speedup vs baseline: 1.0441x; 1.0441x over previous
"""Multi-head (per-task) 2-layer MLP classifier for Trainium2, 8 NeuronCores.

Strategy: expert-parallel with host-side dispatch. Rows of x are grouped by
task_id on the host (the all-to-all "dispatch"); core t gets all rows whose
task_id == t, zero-padded to a fixed PAD columns, pre-transposed to x^T
[D, PAD]. Each core then runs a dense 2-layer MLP for its own head only:

    H^T = relu(W1^T x^T + b1)        [H, PAD]   (psum: out=W1.T@xT, lhsT=W1)
    Y^T = W2^T H^T + b2              [C, PAD]   (lhsT=W2, rhs=H^T)

Everything stays "transposed" (feature dim on partitions, batch on the free
dim) so both matmuls chain without any on-device transpose, and both biases
are per-partition vectors. The host scatters Y^T columns back to the
original row order.

This computes each row through exactly one head (the reference computes all
8 heads and selects via one-hot -> 8x the FLOPs) and reads each expert's
weights from HBM exactly once across the whole chip.

Perf notes (8xNC_v3, NTFF profile; 54.7us -> ~51.5us -> ~50.2us typ):
- Measured timeline (max core): 0-7.2us walrus preamble ($E[4] host
  trigger ~2.8us + two S[2] rendezvous rounds + per-queue TENSOR_LOAD
  queue-init, all fixed); first DMAs issue at kernel entry ~7.25us but
  their completion SEMS fire only ~11.3-12.3us (desc-gen 0.6-0.85us +
  ring/engine pipe; data packets start flowing ~8.2us); PE stream
  ~33.1us of pure matmul cycles (512-col matmul start-pitch is EXACTLY
  216ns = 512cy@2.37GHz, ldweights fully hidden); tail ~2.2-2.5us.
- exec_time_ns ends ~0.5us after the LAST OUTPUT DMA PACKET: the NEFF
  epilogue's 256-sem sweep is mostly EXCLUDED from the measured window
  (revising the earlier ~7us-epilogue theory). Only the last-output
  path matters at the end. The teardown diet (KMM_FASTDRAIN) stays.
- PE warmup: 10 big (512-free) warms bridge entry->gate. The p-state
  ramp needs HIGH-MAC-RATE busy: bridging with 64-free micro-warms
  parked the clock governor at ~2.17GHz for the ENTIRE stream (pitch
  235ns vs 216ns, +3.7us) — only full-width warms are safe. A gap
  between warm-end and gate resets the ramp (~0.6us hiccup + ~2us
  re-ramp at 427ns/matmul), so erring on +1 warm is cheaper.
- w1/xt DMAs alternate across BOTH HWDGE rings in consumption order
  (w1_k and xt_k0 on opposite rings, flipping each k): a single-ring
  w1 stream starves the chunk-0 k-sweep 0.6-2.7us on slow cores
  because ring issue pitch is ~0.65-0.7us/DMA (desc-gen-bound,
  size-independent) and the wait legalizer merges sem waits so
  k-groups gate on w1 tiles ~2 ahead.
- Output: per-chunk CONTIGUOUS DRAM tensors yt0/yt1/yt2 (a column
  slice of one [C,pad] tensor wrote 100x256B strided packets). Tail
  floor ~2.2us = ps2 drain 0.3 + desc-gen ~0.8 (FIXED per DMA, not
  row-proportional) + 100 latency-bound packets ~1.0 + sem margin 0.5.
- PSUM drains (bias+relu) alternate DVE/ACT (GpSimd cannot touch
  PSUM). ~280-320ns each regardless of width (fixed PSUM-read setup).
- Tail chunk L1 runs in m-groups of 2 (KMM_TAILHALF/KMM_TAILG=2,
  quarters): each group's k=7 stops land progressively, so drains +
  L2 matmuls overlap later groups' L1 instead of one drain-paced
  chain at the end. Quarters beat halves (~0.3us) and beat no split
  (~0.4-0.7us); g=1 would serialize ldweights (~1us/m-group, 8us
  tail). Applied ONLY to the last chunk — for chunk 0 a partial
  m-sweep doubles the early DMA demand rate and starves. The tail's
  ps2 drain also splits rows [0:64]/[64:100] across DVE and ACT in
  parallel (engine APs need 32-aligned base partitions — a 50/50
  split crashes codegen).
- The first-DMA completion sems really do fire at ~11-12.3us
  (verified: first real LDWEIGHTS evt_wait 3.8us on S[w1_0]); the
  4-byte sem packets seen at 8.4-9.4us are the SWDGE bias/w2
  completions, NOT the ring tiles. Cutting warms below ~10 on that
  false signal measured +0.5-2us (idle gap 8.4->11 + full re-ramp).
- Dead ends, measured this round: micro-warms (p-state poison, above);
  tail chunk processed mid-stream (final 512-wide write transfers
  204KB serially, worse); tail write split across both rings (desc-gen
  is fixed-cost, keep-warm dummies add overhead, net worse); walrus
  --max-sem-num (bass allocates kernel sems at 151+ itself; and the
  sweep barely counts in the window anyway); DMA directly from PSUM
  (bass asserts src in SBUF/DRAM); late-xt tiles over SWDGE (single
  queue ~50-60GB/s, +3us; plain dma_start can't fan across the 4
  SWDGE queues — only gather/scatter take queue_num).
- Run-to-run variance is ±0.5-1us with occasional +2-3us outliers:
  single-core mid-stream delivery stalls (all 8 cores' rings contend
  for the shared DMA-engine/HBM pool in the 8-20us window at ~2TB/s
  aggregate). Environmental; not addressable from one SPMD program.
- Dead ends from earlier rounds: fp8 fails the 2e-2 gate (3.5e-2;
  DoubleRow=2x only helps if BOTH operands stay plain fp8 — any hi/lo
  compensation burns the pair slot and is >= bf16 cost); walrus
  --enable-ldw-opt rejects tile-emitted InstLdweights; SWDGE for the
  critical output; removing the final drain (~1us worse).
- Remaining structure (don't re-chase): ~0.4-0.5us instruction-fetch
  bubbles at chunk boundaries (1KB iqueue pages); the tail L2 is
  drain/ldweights-paced (~1.2us after last-chunk L1); preamble+DMA
  pipe = ~12.3us fixed front-end. bf16 PE floor for 1088 cols is
  ~33.1us; per-core load is data-given (max task count 1082).
"""

import os

import numpy as np

import concourse.bacc as bacc
import concourse.bass as bass
import concourse.mybir as mybir
import concourse.tile as tile
from concourse.bass_utils import run_bass_kernel_spmd
from concourse.vector_clock import ScopedClock

# Problem constants (nn_MultiHeadClassifier: T tasks, 2-layer MLP heads)
T = 8          # tasks == cores
D = 1024       # d_model
HID = 1024     # hidden
C = 100        # classes
B = 8192       # batch
P = 128        # partitions
KD = D // P    # k-tiles in layer-1 contraction
KH = HID // P  # k-tiles in layer-2 contraction

# Per-core padded batch. Task counts for the graded inputs (seed 0) max
# out at 1082 -> pad 1088; _run grows this automatically if a different
# distribution needs more.
PAD_DEFAULT = 1088

_MM_DTYPES = {
    "f32": mybir.dt.float32,
    "f32r": mybir.dt.float32r,
    "bf16": mybir.dt.bfloat16,
}


def _lean_drain_and_barrier(self, tick_clock, wait_clock):
    """Teardown diet: both all_engine_barriers and the semaphore clear
    are dropped — the walrus NEFF epilogue has its own ordered
    rendezvous chain on S[2] (each queue waits S[2]==k then increments,
    preceded by a walrus-emitted per-queue ring DRAIN) before its full
    256-sem zeroing sweep, so no kernel-side barrier is needed and the
    sweep makes the kernel-side sem clear redundant.

    KMM_FASTDRAIN (default on): the kernel-side drain is emitted WITHOUT
    its global-clock waits, so queues reach the rendezvous at their last
    real instruction and most of the output flush overlaps the ~7us
    sweep. Output consistency holds by a wide margin: the final yt DMA
    lands long before the sweeps finish, walrus's own ring drains cover
    the rings, and the runtime drains them again at teardown; the sweep
    zeroing a DMA-lane semaphore that a late completion then bumps is
    benign — nothing waits on those sems afterward and the next
    execution's preamble re-clears the whole range. (Removing the drain
    instruction entirely measured ~1us WORSE with higher variance.)"""
    drain_inst = self.nc.sync.drain()
    if os.environ.get("KMM_FASTDRAIN", "1") != "1":
        wait_clock.add_sem_waits(
            drain_inst.ins, ScopedClock({None: tick_clock.global_clock})
        )
    popped = self.nc._tile_sem_poison_stack.pop()
    assert popped is self._sem_poison


tile.TileContext._drain_and_barrier = _lean_drain_and_barrier


def _bvo_ldwopt(tmpdir, inp="bir.json", outp="file.neff", arch=None, *,
                dve_root=None):
    """bass_utils.bir_verify_and_optimise with --enable-ldw-opt=true
    (upstream hardcodes false). Elides redundant LDWEIGHTS for
    back-to-back matmuls sharing a stationary operand."""
    import concourse.bass_utils as _bu
    from pathlib import Path as _Path
    cmd = [
        _bu.get_walrus_driver(),
        "--pass",
        ",".join([
            "birverifier", "runtime_memory_reservation", "lower_act",
            "lower_dve", "lower_ap_offset", "codegen", "neff_packager",
        ]),
        "-i", inp,
        "--neff-output-filename", outp,
        "--enable-birsim=true",
        "--mem-mode=physical",
        "--policy=0",
        "--enable-ldw-opt=true",
        "--assign-static-dmas-to-sp=false",
        f"--dram-page-size={_bu.aot_getenv('NEURON_SCRATCHPAD_PAGE_SIZE', '256')}",
        "--enable-neff-debug-info=true",
        "--jobs", "8",
        *_bu.get_walrus_args(
            _bu.get_bir_arch(tmpdir, inp) if arch is None else arch,
            tmpdir, dve_root=dve_root,
        ),
    ]
    result = _bu.run_command(cmd, cwd=tmpdir)
    if result is not None:
        (_Path(tmpdir) / "log.txt").write_text(result.stdout)
    return f"{tmpdir}/{outp}"


# Dead end, kept for reference: walrus rejects the tile-emitted explicit
# InstLdweights with "not compatible with LDW optimization".
if os.environ.get("KMM_LDWOPT", "0") == "1":
    import concourse.bass_utils as _bu_mod
    _bu_mod.bir_verify_and_optimise = _bvo_ldwopt


def _bvo_extra_flags(extra):
    """Wrap bass_utils.bir_verify_and_optimise, appending walrus flags."""
    import concourse.bass_utils as _bu
    import subprocess as _sp
    orig_run = _bu.run_command

    def patched_run(cmd, **kw):
        if cmd and "walrus_driver" in str(cmd[0]):
            cmd = list(cmd) + extra
        return orig_run(cmd, **kw)

    _bu.run_command = patched_run


# Experiment: cap walrus's semaphore allocation. The NTFF "useful window"
# (what exec_time_ns measures) ends when the epilogue's ascending per-queue
# 256-sem zeroing sweep clears the last semaphore id the kernel used;
# fewer/lower sem ids end the window sooner.
if os.environ.get("KMM_SEMCAP"):
    _bvo_extra_flags([f"--max-sem-num={os.environ['KMM_SEMCAP']}"])


def _chunks(total, size=512):
    out, o = [], 0
    while o < total:
        c = min(size, total - o)
        out.append((o, c))
        o += c
    return out


def build_program(pad, mm_dtype="bf16"):
    """One SPMD NeuronCore program: dense 2-layer MLP on [D, pad] x^T."""
    mm_dt = _MM_DTYPES[mm_dtype]
    f32 = mybir.dt.float32
    # Tensors consumed by the matmuls carry the matmul dtype end-to-end
    # (walrus requires fp32r-consumed buffers to be *produced* as fp32r).
    io_dt = mm_dt

    def mm(ap):
        return ap.bitcast(mm_dt) if ap.dtype != mm_dt else ap

    nc = bacc.Bacc()
    # Hybrid-precision L1 contraction (KMM_FP8PAIRS, default OFF): put
    # 2*pairs k-slices in ONE fp8e4 DoubleRow matmul per (chunk, m) —
    # the PE packs 2 k-slices per instruction at bf16 speed — while the
    # rest stay bf16. MEASURED at 1 pair (minimum coverage): 47509ns
    # (saves ~2.9us) but rel err 2.1417e-2 — 7% OVER the 2e-2 gate
    # (error scales as sqrt(coverage); full-fp8 extrapolates to 4.3e-2;
    # W-side/x-side split is ~55/45 so no single-side fix reaches 2e-2,
    # and every compensation scheme costs exactly the 2x DoubleRow
    # saves: the pair slot must hold the correction). Kept for a future
    # relaxed gate; enable with KMM_FP8PAIRS=1.
    fp8_pairs = int(os.environ.get("KMM_FP8PAIRS", "0"))
    assert fp8_pairs * 2 <= KD
    kd_b = KD - 2 * fp8_pairs  # bf16 k-slices
    fp8 = mybir.dt.float8e4
    d_split = fp8_pairs * 2 * P  # first d rows in fp8
    if fp8_pairs:
        w1f = nc.dram_tensor("w1f", [P, fp8_pairs * 2 * HID], fp8,
                             kind="ExternalInput")
        xtf = nc.dram_tensor("xtf", [P, fp8_pairs * 2 * pad], fp8,
                             kind="ExternalInput")
    xt = nc.dram_tensor("xt", [kd_b * P, pad], io_dt, kind="ExternalInput")
    w1 = nc.dram_tensor("w1", [kd_b * P, HID], io_dt, kind="ExternalInput")
    b1 = nc.dram_tensor("b1", [P, KH], f32, kind="ExternalInput")
    w2 = nc.dram_tensor("w2", [HID, P], io_dt, kind="ExternalInput")
    b2 = nc.dram_tensor("b2", [C, 1], f32, kind="ExternalInput")
    chunks = _chunks(pad)
    # Per-chunk output tensors: each chunk's [C, cw] block is CONTIGUOUS in
    # DRAM (rows of cw*4B at stride cw*4B), so the final write coalesces
    # into large packets instead of the 256B-per-row strided writes a
    # column-slice of a single [C, pad] tensor produces (measured: the
    # 64-col tail chunk wrote 100 x 256B packets over ~1.3us).
    yts = [nc.dram_tensor(f"yt{ci}", [C, cw], f32, kind="ExternalOutput")
           for ci, (_, cw) in enumerate(chunks)]
    # (KMM_TAILSPLIT=1 — splitting the tail write across both rings with
    # keep-warm dummies — was measured neutral-to-worse: DMA desc-gen is
    # ~0.8us FIXED per instruction regardless of row count, so the split
    # only parallelizes ~0.5us of packets while the dummies add overhead.)
    tailsplit = os.environ.get("KMM_TAILSPLIT", "0") == "1"
    ydummy = (nc.dram_tensor("ydummy", [1, 4], f32, kind="Internal")
              if tailsplit else None)

    w1_t = w1.rearrange("(k p) h -> k p h", p=P)
    xt_t = xt.rearrange("(k p) b -> k p b", p=P)
    w2_t = w2.rearrange("(k p) c -> k p c", p=P)

    with tile.TileContext(nc) as tc:
        with (
            tc.tile_pool(name="weights", bufs=1) as wpool,
            tc.tile_pool(name="acts", bufs=1) as apool,
            tc.tile_pool(name="ps", bufs=8, space="PSUM") as pspool,
            tc.tile_pool(name="outs", bufs=3) as opool,
        ):
            # DMA plan. The critical first tiles (w1 k=0 halves, xt chunk-0
            # k-tiles) ride the SP HWDGE ring, issued FIRST and interleaved
            # so PE's chunk-0 k-sweep consumes each (w1_k, xt_k0) as it
            # lands; odd w1 k-tiles ride the Activation HWDGE ring (which
            # starts late: the RELU act-table load is queued ahead of it).
            # Small/late tensors (biases, w2, output) ride SWDGE (gpsimd).
            b1_sb = wpool.tile([P, KH], f32, name="b1", tag="b1")
            nc.gpsimd.dma_start(out=b1_sb[:], in_=b1[:])
            b2_sb = wpool.tile([C, 1], f32, name="b2", tag="b2")
            nc.gpsimd.dma_start(out=b2_sb[:], in_=b2[:])

            w2_all = wpool.tile([P, KH, P], io_dt, name="w2_all", tag="w2_all")
            nc.gpsimd.dma_start(
                out=w2_all[:],
                in_=w2.rearrange("(k p) c -> p k c", p=P),
            )
            w2_sb = [w2_all[:, k, :] for k in range(KH)]


            # xt tile plan: one tile per (k, big-chunk); a trailing small
            # chunk (pad % 512) is folded into the previous tile so it
            # needs no extra DMA. (Merging each k's xt into one [P, pad]
            # tile was measured WORSE: pad*2B rows cross the 2KB
            # descriptor boundary and split into an inefficient 2nd
            # descriptor per row.) chunk_src[ci] = (tile index, offset).
            tiles = []          # (start, width)
            chunk_src = []      # ci -> (tile_idx, offset)
            for (o, cw) in chunks:
                if tiles and cw < 128:
                    ti, (ts, tw) = len(tiles) - 1, tiles[-1]
                    chunk_src.append((ti, o - ts))
                    tiles[ti] = (ts, tw + cw)
                else:
                    chunk_src.append((len(tiles), 0))
                    tiles.append((o, cw))

            w1_sb = [None] * KD
            xt_sb = [[None] * len(tiles) for _ in range(KD)]

            def load_w1(k, q):
                w1_sb[k] = wpool.tile([P, HID], io_dt, name=f"w1_{k}",
                                      tag=f"w1_{k}")
                q.dma_start(out=w1_sb[k][:], in_=w1_t[k, :, :])

            def load_xt(k, ti, q):
                ts, tw = tiles[ti]
                t = wpool.tile([P, tw], io_dt, name=f"xt_{k}_{ti}",
                               tag=f"xt_{k}_{ti}")
                q.dma_start(out=t[:], in_=xt_t[k, :, ts:ts + tw])
                xt_sb[k][ti] = t

            # DMA order (trace-measured pipe latencies: the sync/SP ring's
            # first DMA lands ~4.0us after issue, the scalar/ACT ring's
            # ~5.0us — sync arms ~1us earlier). The first-matmul gate is
            # (w1_0, xt_00): put w1_0 FIRST on sync (lands ~11.6us) and
            # xt_00 FIRST on scalar (~11.9us) so neither serializes behind
            # the other on one ring. Splitting w1_0 finer is useless: ring
            # issue pitch (~0.7us/DMA descriptor-gen) dominates, and the
            # wait-legalization hoists the k-group's waits anyway.
            # Interleave w1/xt across BOTH rings in consumption order. With
            # w1 all on one ring (0.65-0.7us/DMA issue pitch) the chunk-0
            # k-sweep starves 0.6-1.5us at ~16-17us on most cores: the
            # wait legalizer merges sem waits so k-group k gates on w1
            # tiles ~2 ahead of k, and a single ring delivers w1_7 only by
            # ~17.3us. Alternating (w1_k, xt_k0) pairs across the rings
            # delivers each stream at 2x rate in need order.
            if fp8_pairs:
                w1f_sb = wpool.tile([P, fp8_pairs * 2 * HID], fp8,
                                    name="w1f", tag="w1f")
                nc.scalar.dma_start(out=w1f_sb[:], in_=w1f[:])
                xtf_sb = wpool.tile([P, fp8_pairs * 2 * pad], fp8,
                                    name="xtf", tag="xtf")
                nc.sync.dma_start(out=xtf_sb[:], in_=xtf[:])
                w1f_ap = w1f_sb[:].rearrange(
                    "p (two h) -> p two h", two=fp8_pairs * 2)
                xtf_ap = xtf_sb[:].rearrange(
                    "p (two b) -> p two b", two=fp8_pairs * 2)
            # Balanced first-group split (KMM_SPLIT0): the gate is the
            # max over both rings' first transfers; with w1_0 (256KB) on
            # one ring and xt_00 (128KB) on the other it is bound by the
            # 256KB ring. Halving BOTH across BOTH rings makes each
            # ring's first two items 192KB, and k0's m-sweep consumes
            # (w1 half, xt col-half) pieces as they land.
            # (KMM_SPLIT0=1 measured ~0.5us WORSE on interleaved A/B:
            # the 2 extra ring-head items delay every later k-tile by a
            # ~0.65us desc-gen slot, and the earlier gate is wasted
            # because the 10-warm chain still binds the stream start.)
            split0 = (os.environ.get("KMM_SPLIT0", "0") == "1"
                      and chunks[0][1] == 512)
            if split0:
                w1_0a = wpool.tile([P, 512], io_dt, name="w1_0a",
                                   tag="w1_0a")
                nc.scalar.dma_start(out=w1_0a[:], in_=w1_t[0, :, 0:512])
                xt_00a = wpool.tile([P, 256], io_dt, name="xt_00a",
                                    tag="xt_00a")
                nc.scalar.dma_start(out=xt_00a[:], in_=xt_t[0, :, 0:256])
                w1_0b = wpool.tile([P, 512], io_dt, name="w1_0b",
                                   tag="w1_0b")
                nc.sync.dma_start(out=w1_0b[:], in_=w1_t[0, :, 512:1024])
                xt_00b = wpool.tile([P, 256], io_dt, name="xt_00b",
                                    tag="xt_00b")
                nc.sync.dma_start(out=xt_00b[:], in_=xt_t[0, :, 256:512])
            for k in range(kd_b):
                if k == 0 and split0:
                    continue
                load_w1(k, nc.scalar if k % 2 == 0 else nc.sync)
                load_xt(k, 0, nc.sync if k % 2 == 0 else nc.scalar)
            # xt tiles for chunks >= 1 are not consumed until ~26us; route
            # them over the SWDGE (gpsimd) queue so the two HWDGE rings
            # carry only w1 + chunk-0 xt (3MB instead of 4.2MB) and finish
            # chunk-0's k-tail ~1us sooner on congested cores (measured
            # 1.3us delivery stalls at ~17us with everything on HWDGE).
            # (KMM_XTSW=1 — routing them over SWDGE — was measured ~3us
            # WORSE on every core: the single SWDGE queue delivers only
            # ~50-60GB/s, starving chunks 1-2; plain dma_start cannot
            # spread across the 4 SWDGE queues.)
            xt_late_q = nc.gpsimd if os.environ.get(
                "KMM_XTSW", "0") == "1" else None
            for ti in range(1, len(tiles)):
                for k in range(kd_b):
                    load_xt(k, ti, xt_late_q or
                            (nc.sync if k % 2 == 0 else nc.scalar))

            # PE warmup: the PE clock ramps over ~3us of sustained
            # activity (cold matmuls run at ~427ns vs 216ns full speed for
            # 512-free), and the ramp is TIME-based while busy — an idle
            # gap between warm-end and the first real matmul RESETS the
            # ramp (measured: a 1.1us gap cost ~2.3us of re-ramp slowdown
            # on top of the gap itself). Bridge from PE-queue start
            # (~7.4us) to the first real matmul's DMA gate (~11.9-12.4us)
            # with 7 big warms (512-free, ~427ns cold) + fine-grained
            # micro-warms (64-free) whose overshoot granularity is small.
            # NOTE: bridging with fine-grained 64-free micro-warms was
            # measured ~3.7us WORSE: the low-MAC-rate micro-warm window
            # parks the PE clock governor in a mid p-state (~2.17GHz) for
            # the ENTIRE stream (512-free matmul pitch 235ns vs 216ns,
            # durations 379->413ns). Only big high-duty warms keep the
            # ramp going.
            warm = wpool.tile([P, 512], io_dt, name="warm", tag="warm")
            nc.vector.memset(warm[:], 0.0)
            for _ in range(int(os.environ.get("KMM_WARM", "10"))):
                pw = pspool.tile([P, 512], f32, name="ps_w", tag="ps")
                nc.tensor.matmul(out=pw[:], lhsT=mm(warm[:, 0:P]),
                                 rhs=mm(warm[:]), start=True, stop=True)

            def w1_block(k, m):
                if k == 0 and split0:
                    half = w1_0a if m < KH // 2 else w1_0b
                    mo = m % (KH // 2)
                    return half[:, mo * P:(mo + 1) * P]
                return w1_sb[k][:, m * P:(m + 1) * P]

            h_sb = [apool.tile([P, pad], io_dt, name=f"h_{m}", tag=f"h_{m}") for m in range(KH)]

            # Process the small tail chunk MID-stream (order [0, last,
            # middles]): its latency-bound output (100x256B packets, ~1us)
            # and drain-paced L2 hide under later compute, and the final
            # chunk becomes a 512-wide one whose output writes efficient
            # 2KB packets.
            order = list(range(len(chunks)))
            # (KMM_TAILMID=1 — processing the small chunk mid-stream so a
            # 512 chunk lands last — was measured NEUTRAL-to-worse: the
            # final 204KB write transfers ~2.3us serially, vs the 64-col
            # tail's latency-bound ~1.9us.)
            if len(chunks) > 2 and os.environ.get("KMM_TAILMID", "0") == "1":
                order = [0, len(chunks) - 1] + order[1:-1]
            for ci in order:
                o, cw = chunks[ci]
                ti, toff = chunk_src[ci]
                # layer 1: all KH h-tile groups resident in PSUM, k swept in
                # the middle so PE consumes (w1_k, xt_k) right as each DMA
                # lands instead of stalling a single group on the last tile.
                pss = [pspool.tile([P, 512], f32, name=f"ps_{m}", tag="ps")
                       for m in range(KH)]

                def l1_sweep(ms):
                    # fp8 DoubleRow pair(s) first (start=True zeroes the
                    # psum region), then the bf16 k-slices accumulate.
                    # k swept in the middle so PE consumes (w1_k, xt_k)
                    # right as each DMA lands instead of stalling a single
                    # group on the last tile.
                    for pr in range(fp8_pairs):
                        for m in ms:
                            nc.tensor.matmul(
                                out=pss[m][:, :cw],
                                lhsT=w1f_ap[:, 2*pr:2*pr+2,
                                            m * P:(m + 1) * P],
                                rhs=xtf_ap[:, 2*pr:2*pr+2, o:o + cw],
                                start=(pr == 0),
                                stop=False,
                                perf_mode=mybir.MatmulPerfMode.DoubleRow,
                            )
                    for k in range(kd_b):
                        for m in ms:
                            if k == 0 and ci == 0 and split0 \
                                    and not fp8_pairs:
                                # col-halves as the half-tiles land;
                                # start=True zeroes the whole 2KB psum
                                # bank region, so half B accumulates
                                # with start=False onto pending-zero.
                                nc.tensor.matmul(
                                    out=pss[m][:, 0:256],
                                    lhsT=mm(w1_block(0, m)),
                                    rhs=mm(xt_00a[:]),
                                    start=True, stop=False,
                                )
                                nc.tensor.matmul(
                                    out=pss[m][:, 256:512],
                                    lhsT=mm(w1_block(0, m)),
                                    rhs=mm(xt_00b[:]),
                                    start=False, stop=False,
                                )
                                continue
                            nc.tensor.matmul(
                                out=pss[m][:, :cw],
                                lhsT=mm(w1_block(k, m)),
                                rhs=mm(xt_sb[k][ti][:, toff:toff + cw]),
                                start=(k == 0 and not fp8_pairs
                                       and not (ci == 0 and split0)),
                                stop=(k == kd_b - 1),
                            )

                def drains(ms):
                    # PSUM -> SBUF h with bias+relu, alternating DVE/ACT so
                    # the drains take half as many op-slots of wall time.
                    # (GpSimd cannot read PSUM.)
                    for m in ms:
                        if m % 2 == 0:
                            nc.vector.tensor_scalar(
                                out=h_sb[m][:, o:o + cw],
                                in0=pss[m][:, :cw],
                                scalar1=b1_sb[:, m:m + 1],
                                scalar2=0.0,
                                op0=mybir.AluOpType.add,
                                op1=mybir.AluOpType.max,
                            )
                        else:
                            nc.scalar.activation(
                                out=h_sb[m][:, o:o + cw],
                                in_=pss[m][:, :cw],
                                func=mybir.ActivationFunctionType.Relu,
                                bias=b1_sb[:, m:m + 1],
                                scale=1.0,
                            )

                if ci == order[-1] and os.environ.get(
                        "KMM_TAILHALF", "1") == "1":
                    # Tail-critical chunk: run L1 in m-groups so early
                    # groups' k=7 stops land mid-tail — their drains and
                    # L2 matmuls overlap later groups' L1 instead of a
                    # single drain-paced chain at the end (measured ~1.2us
                    # from tail-L1 end to last L2). Group size via
                    # KMM_TAILG (2 = quarters, 4 = halves).
                    g = int(os.environ.get("KMM_TAILG", "2"))
                    for lo in range(0, KH, g):
                        l1_sweep(range(lo, lo + g))
                        drains(range(lo, lo + g))
                else:
                    l1_sweep(range(KH))
                    drains(range(KH))
                # layer 2: Y^T chunk = sum_k W2[k].T @ H^T[k] + b2
                ps2 = pspool.tile([P, 512], f32, name="ps2", tag="ps")
                for k in range(KH):
                    nc.tensor.matmul(
                        out=ps2[:, :cw],
                        lhsT=mm(w2_sb[k]),
                        rhs=mm(h_sb[k][:, o:o + cw]),
                        start=(k == 0),
                        stop=(k == KH - 1),
                    )
                ot = opool.tile([P, 512], f32, name="ot", tag="ot")
                if ci == order[-1]:
                    # Tail-critical ps2 drain: split across DVE and ACT in
                    # parallel (~halves the ~270ns drain on the chain).
                    # Split at 64: engine APs need 32-aligned base
                    # partitions.
                    c2 = 64
                    nc.vector.tensor_scalar_add(
                        out=ot[:c2, :cw],
                        in0=ps2[:c2, :cw],
                        scalar1=b2_sb[:c2, 0:1],
                    )
                    nc.scalar.activation(
                        out=ot[c2:C, :cw],
                        in_=ps2[c2:C, :cw],
                        func=mybir.ActivationFunctionType.Identity,
                        bias=b2_sb[c2:C, 0:1],
                        scale=1.0,
                    )
                else:
                    nc.vector.tensor_scalar_add(
                        out=ot[:C, :cw],
                        in0=ps2[:C, :cw],
                        scalar1=b2_sb[:, 0:1],
                    )
                if ci == order[-1] and tailsplit:
                    # Tail-critical write: split by partition rows across
                    # BOTH rings so desc-gen (~0.45us each for 50 rows) and
                    # the latency-bound 256B row packets run in parallel.
                    # The scalar ring is kept warm by the dummy DMAs below
                    # (a COLD scalar ring measured ~2us re-arm, which is
                    # why a split without keep-warm dummies lost ~0.5us).
                    c2 = (C + 1) // 2
                    nc.sync.dma_start(out=yts[ci][0:c2, :cw],
                                      in_=ot[0:c2, :cw])
                    nc.scalar.dma_start(out=yts[ci][c2:C, :cw],
                                        in_=ot[c2:C, :cw])
                else:
                    # yt rides the sync queue: each chunk's write follows
                    # the previous one on a HOT ring.
                    nc.sync.dma_start(out=yts[ci][:, :cw], in_=ot[:C, :cw])
                    if tailsplit:
                        # Keep-warm dummy on the scalar ring (~4B).
                        nc.scalar.dma_start(out=ydummy[0:1, 0:2],
                                            in_=ot[0:1, 0:2])
    return nc


def _pad_cols(a, n):
    out = np.zeros((a.shape[0], n), dtype=a.dtype)
    out[:, :a.shape[1]] = a
    return out


def _route(task_id):
    """Group rows by task. Returns (row-index list per task, counts)."""
    task_id = np.asarray(task_id)
    order = np.argsort(task_id, kind="stable")
    counts = np.bincount(task_id.astype(np.int64), minlength=T)
    offs = np.zeros(T + 1, dtype=np.int64)
    np.cumsum(counts, out=offs[1:])
    rows = [order[offs[t]:offs[t + 1]] for t in range(T)]
    return rows, counts


def _run(inputs, trace=False):
    x = np.ascontiguousarray(np.asarray(inputs["x"], dtype=np.float32))
    task_id = np.asarray(inputs["task_id"])
    W1 = np.asarray(inputs["W1"], dtype=np.float32)
    b1 = np.asarray(inputs["b1"], dtype=np.float32)
    W2 = np.asarray(inputs["W2"], dtype=np.float32)
    b2 = np.asarray(inputs["b2"], dtype=np.float32)

    mm_dtype = os.environ.get("KMM_DTYPE", "bf16")
    pad = int(os.environ.get("KMM_PAD", PAD_DEFAULT))
    rows, counts = _route(task_id)
    if counts.max() > pad:  # unexpected distribution: grow pad to fit
        pad = int(-(-int(counts.max()) // 16) * 16)

    io_np = np.float32
    if mm_dtype == "bf16":
        import ml_dtypes
        io_np = ml_dtypes.bfloat16

    fp8_pairs = int(os.environ.get("KMM_FP8PAIRS", "0"))
    d_split = fp8_pairs * 2 * P
    fp8_np = None
    if fp8_pairs:
        import ml_dtypes
        fp8_np = ml_dtypes.float8_e4m3

    def _pack_fp8(a):
        # [d_split, N] (d = slice*128 + p) -> [128, n_slices*N] fp8 with
        # per-partition layout [slice, N] (matches the [p, two, N] APs).
        n = a.shape[1]
        return np.ascontiguousarray(
            a.reshape(d_split // P, P, n).transpose(1, 0, 2).reshape(
                P, d_split // P * n)).astype(fp8_np)

    in_maps = []
    for t in range(T):
        xt = np.zeros((D, pad), dtype=np.float32)
        xt[:, :counts[t]] = x[rows[t]].T
        w1t = np.ascontiguousarray(W1[t])
        im = {
            "xt": xt[d_split:].astype(io_np),
            "w1": w1t[d_split:].astype(io_np),
            "b1": np.ascontiguousarray(b1[t].reshape(KH, P).T.astype(np.float32)),
            "w2": _pad_cols(W2[t], P).astype(io_np),
            "b2": np.ascontiguousarray(b2[t][:, None].astype(np.float32)),
        }
        if fp8_pairs:
            im["w1f"] = _pack_fp8(w1t[:d_split])
            im["xtf"] = _pack_fp8(xt[:d_split])
        in_maps.append(im)

    nc = build_program(pad, mm_dtype)
    nc.finalize()  # Bacc passes: legalize sync waits (<=1 per instruction)
    res = run_bass_kernel_spmd(
        nc, in_maps, core_ids=list(range(T)), trace=trace,
        trace_cores=list(range(T)) if trace else None,
        tmpdir=os.environ.get("KMM_TMPDIR"),
    )

    chunks = _chunks(pad)
    out = np.empty((task_id.shape[0], C), dtype=np.float32)
    for t in range(T):
        ytf = np.concatenate(
            [res.results[t][f"yt{ci}"] for ci in range(len(chunks))], axis=1)
        out[rows[t]] = ytf[:, :counts[t]].T
    return out, res


def kernel(**inputs):
    out, _ = _run(inputs, trace=False)
    return out

